# revision 14
# baseline (speedup 1.0000x reference)
"""Causal attention kernel for TRN2, sharded over 8 NeuronCores.

Problem: q,k,v [B=2, H=16, S=2048, D=64] fp32 -> causal softmax(QK^T)V.
Sharding: 32 (batch*head) pairs -> 4 heads per core (head-parallel, no comm).

Per-core algorithm (per head), v2 -- engine-balanced:
  Scores are computed TRANSPOSED per 512-wide q-chunk: st[k, q] via
  matmul(out=sc, lhsT=KT[d, k-tile], rhs=QT[d, q-chunk-slice]), k-tiles
  processed in pairs sharing one [128, 1024] PSUM tile (fp32r, all output
  pieces >= 256 wide so the PE streams 1 col/cycle).
  exp() is split across THREE engines per stage:
    - ACT: exact exp (bf16 out), one strided instruction covering both
      tiles' ACT column shares.
    - DVE + Pool (gpsimd): Schraudolph exp -- one scalar_tensor_tensor
      computing round_i16(score * 128*log2(e) + bias) written through the
      bf16 ex tile's int16 bitcast view; the int16 bits ARE the bf16
      encoding of 2^(x*log2 e) ~ exp(x) (~3% max rel err; tolerance 2e-2).
      The causal mask is FOLDED into the same op: the in1 bias tile holds
      (bias - 3e5) on disallowed diagonal lanes, so the int16 convert
      saturates negative -> bitcast -0.0 -> contributes exactly 0.
  mm2 is q-on-partitions: matmul(acc[q-block 128, 65] +=,
  lhsT=ex[k-tile, q-block], rhs=[V|1][k-tile, 65] bf16): 65-wide moving
  operand at 1 col/cycle (bf16) -- half the PE cost of the 512-wide
  accT formulation -- and the softmax denominator Z lands in acc column
  64 per PARTITION, so normalization is a native per-partition scalar:
  reciprocal_approx_fast on the strided Z view + one broadcast multiply.
  Output is emitted per chunk as o[h][p, qblock, d] (q = 128*qblock + p);
  host reorders.
"""

import numpy as np
import ml_dtypes

import concourse.bacc as bacc
import concourse.tile as tile
from concourse import mybir
from concourse import bass_utils

B, H, S, D = 2, 16, 2048, 64
N_CORES = 8
HPC = (B * H) // N_CORES  # heads per core = 4
F32 = mybir.dt.float32
F32R = mybir.dt.float32r  # full-rate PE matmul mode (TF32-like rounding)
BF16 = mybir.dt.bfloat16
I16 = mybir.dt.int16
Alu = mybir.AluOpType
KT_N = S // 128  # 16 k-tiles per head
MV = D + 1  # V columns + ones column
NCH = 4  # 512-wide q-chunks per head

LOG2E = float(np.log2(np.e))
SCH_SCALE = LOG2E * 128.0
SCH_BIAS = 127.0 * 128.0 - 5.5  # optimal-C Schraudolph bias (bf16 bits)
MASK_NEG = -3.0e5  # saturates int16 negative -> bf16 -0.0

FA = 400  # ACT column share of each full 512 tile segment

_CACHE = {}


def _build_nc(reps: int = 1):
    """Build + compile the SPMD bass program (same for every core)."""
    nc = bacc.Bacc(
        "TRN2", target_bir_lowering=False, debug=False, num_devices=N_CORES
    )
    qt = nc.dram_tensor("qt", [HPC, D, S], F32R, kind="ExternalInput").ap()
    kt = nc.dram_tensor("kt", [HPC, D, S], F32R, kind="ExternalInput").ap()
    vp = nc.dram_tensor("vp", [HPC, 128, KT_N * MV], BF16, kind="ExternalInput").ap()
    mbw = nc.dram_tensor("mbw", [128, 512], F32, kind="ExternalInput").ap()
    o = nc.dram_tensor("o", [HPC, 128, KT_N, D], F32, kind="ExternalOutput").ap()

    with tile.TileContext(nc) as tc:
        with (
            tc.tile_pool(name="io", bufs=3) as io_pool,
            tc.tile_pool(name="maskp", bufs=1) as mask_pool,
            tc.tile_pool(name="expp", bufs=20) as exp_pool,
            tc.tile_pool(name="outp", bufs=3) as out_pool,
            tc.tile_pool(name="zp", bufs=4) as z_pool,
            tc.tile_pool(name="scores", bufs=3, space="PSUM") as sc_pool,
            tc.tile_pool(name="acc", bufs=2, space="PSUM") as acc_pool,
        ):
            mbw_s = mask_pool.tile([128, 512], F32)

            io_of = {}  # (rep, h) -> (qt_s, kt_s, vp_s)

            def get_io(rep, h):
                if (rep, h) not in io_of:
                    qt_s = io_pool.tile([D, S], F32R, tag="qt")
                    kt_s = io_pool.tile([D, S], F32R, tag="kt")
                    vp_s = io_pool.tile([128, KT_N * MV], BF16, tag="vp")
                    if rep == 0 and h == 0:
                        # staged loads ordered by first use (HWDGE is serial)
                        nc.sync.dma_start(kt_s[:, 0:256], kt[h][:, 0:256])
                        nc.sync.dma_start(qt_s[:, 0:512], qt[h][:, 0:512])
                        nc.sync.dma_start(mbw_s[:], mbw[:])
                        nc.sync.dma_start(vp_s[:], vp[h])
                        nc.sync.dma_start(kt_s[:, 256:1024], kt[h][:, 256:1024])
                        nc.sync.dma_start(qt_s[:, 512:S], qt[h][:, 512:S])
                        nc.sync.dma_start(kt_s[:, 1024:S], kt[h][:, 1024:S])
                    else:
                        nc.sync.dma_start(kt_s[:], kt[h])
                        nc.sync.dma_start(qt_s[:], qt[h])
                        nc.sync.dma_start(vp_s[:], vp[h])
                    io_of[(rep, h)] = (qt_s, kt_s, vp_s)
                return io_of[(rep, h)]

            acc_of = {}  # (rep, h, c) -> acc tile [128, 4*65]

            # ---- stage emission -------------------------------------------
            # stage = (rep, h, c, s): chunk c covers q [512c, 512c+512),
            # pair s covers k-tiles (2s, 2s+1); s in 0..2c+1.

            def emit_mm1(rep, h, c, s):
                qt_s, kt_s, vp_s = get_io(rep, h)
                if (rep, h, c) not in acc_of:
                    acc_of[(rep, h, c)] = acc_pool.tile(
                        [128, 4 * MV], F32, tag="acc", name=f"acc_{rep}_{h}_{c}"
                    )
                sc = sc_pool.tile([128, 1024], F32, tag="sc")
                for ti, t in enumerate((2 * s, 2 * s + 1)):
                    ls = max(0, 128 * t - 512 * c)  # local q start
                    ms = min(ls, 256)  # >=256-wide piece for fp32r full rate
                    base = 512 * ti
                    nc.tensor.matmul(
                        sc[:, base + ms : base + 512],
                        lhsT=kt_s[:, 128 * t : 128 * t + 128],
                        rhs=qt_s[:, 512 * c + ms : 512 * c + 512],
                        start=True,
                        stop=True,
                    )
                return sc

            def emit_exp(rep, h, c, s, sc):
                """Split exp across ACT (exact) and DVE (Schraudolph).

                ACT and DVE write SEPARATE bf16 tiles (shared-tile writes get
                spuriously serialized by bounding-box subtile dep tracking);
                ownership is 128-col-block aligned so each mm2 lhsT slice
                lives wholly in one tile.  Returns (ex_a, ex_d, owners) where
                owners[bi] is True if block bi (col bi*128) is ACT's."""
                ex_a = exp_pool.tile([128, 1024], BF16, tag="exa")
                ex_d = exp_pool.tile([128, 1024], BF16, tag="exd")
                owners = [False] * 8

                def act(lo, w):
                    for bi in range(lo // 128, (lo + w) // 128):
                        owners[bi] = True
                    nc.scalar.activation(
                        ex_a[:, lo : lo + w],
                        sc[:, lo : lo + w],
                        mybir.ActivationFunctionType.Exp,
                    )

                def stt(lo, w, masked):
                    # DVE Schraudolph (GPSIMD cannot access PSUM)
                    boff = 0 if masked else 128
                    nc.vector.scalar_tensor_tensor(
                        ex_d[:, lo : lo + w].bitcast(I16),
                        sc[:, lo : lo + w],
                        SCH_SCALE,
                        mbw_s[:, boff : boff + w],
                        Alu.mult,
                        Alu.add,
                    )

                if s < 2 * c:  # full pair: ACT window alternates sides
                    w = 896 if s % 4 < 2 else 768
                    if s % 2 == 0:
                        act(0, w)
                        stt(w, 1024 - w, False)
                    else:
                        stt(0, 1024 - w, False)
                        act(1024 - w, w)
                elif s == 2 * c:  # tiles 4c (diag at 0), 4c+1 (diag at 128)
                    stt(0, 512, True)  # whole tile0, mask at block 0
                    stt(512 + 128, 128, True)  # tile1 diag block
                    act(512 + 256, 256)
                else:  # s == 2c+1: tiles 4c+2 (diag at 256), 4c+3 (diag at 384)
                    stt(256, 256, True)
                    stt(512 + 384, 128, True)
                return ex_a, ex_d, owners

            ex_of = {}  # (rep, h, c, s) -> ex tile (alive for the chunk)

            def emit_mm2_block(rep, h, c, b):
                """Emit q-block b's FULL accumulation chain consecutively.

                PSUM accumulation groups sharing a bank must not interleave
                (hw corrupts the early-stopping group), so each block's
                matmuls run back-to-back once all its k-tiles' exp is done."""
                _, _, vp_s = get_io(rep, h)
                acc = acc_of[(rep, h, c)]
                tmax = 4 * c + b
                for t in range(tmax + 1):
                    ex_a, ex_d, owners = ex_of[(rep, h, c, t // 2)]
                    bi = 4 * (t % 2) + b
                    ex = ex_a if owners[bi] else ex_d
                    nc.tensor.matmul(
                        acc[:, MV * b : MV * b + MV],
                        lhsT=ex[:, 128 * bi : 128 * bi + 128],
                        rhs=vp_s[:, MV * t : MV * t + MV],
                        start=(t == 0),
                        stop=(t == tmax),
                    )

            def do_norm(rep, h, c):
                acc = acc_of.pop((rep, h, c))
                zr = z_pool.tile([128, 4], F32, tag="zr")
                nc.vector.reciprocal_approx_fast(
                    zr[:], acc[:, D : 4 * MV : MV]
                )
                o_s = out_pool.tile([128, 4 * D], F32, tag="os")
                accv = acc[:].rearrange("p (b c) -> p b c", b=4)
                nc.vector.tensor_tensor(
                    o_s[:].rearrange("p (b c) -> p b c", b=4),
                    accv[:, :, 0:D],
                    zr[:].unsqueeze(2).broadcast_to([128, 4, D]),
                    Alu.mult,
                )
                nc.sync.dma_start(o[h][:, 4 * c : 4 * c + 4, :], o_s[:])

            # ---- flattened software pipeline ------------------------------
            stages = []
            for _rep in range(reps):
                for h in range(HPC):
                    for c in range(NCH):
                        for s in range(2 * c + 2):
                            stages.append((_rep, h, c, s))

            def compute_stage(pst, psc):
                rep_, h_, c_, s_ = pst
                ex_of[(rep_, h_, c_, s_)] = emit_exp(*pst, psc)
                if s_ == 2 * c_:
                    emit_mm2_block(rep_, h_, c_, 0)
                    emit_mm2_block(rep_, h_, c_, 1)
                elif s_ == 2 * c_ + 1:
                    emit_mm2_block(rep_, h_, c_, 2)
                    emit_mm2_block(rep_, h_, c_, 3)
                    for s2 in range(2 * c_ + 2):
                        del ex_of[(rep_, h_, c_, s2)]
                    return [(rep_, h_, c_)]
                return []

            # mm1 runs TWO stages ahead of exp/mm2 so the next chunk's
            # scores are already in PSUM while this chunk's block chains
            # and norm drain -- keeps ACT/DVE fed across chunk boundaries.
            pending = []  # [(stage, sc)] not yet computed
            deferred = []  # chunk norms deferred one stage
            for i, st in enumerate(stages):
                rep_, h_, c_, s_ = st
                if c_ == NCH - 2 and s_ == 0:
                    nxt = (rep_, h_ + 1) if h_ + 1 < HPC else (rep_ + 1, 0)
                    if nxt[0] < reps:
                        get_io(*nxt)
                pending.append((st, emit_mm1(*st)))
                if len(pending) > 2:
                    norms = compute_stage(*pending.pop(0))
                    for n in deferred:
                        do_norm(*n)
                    deferred = norms
            for p in pending:
                norms = compute_stage(*p)
                for n in deferred:
                    do_norm(*n)
                deferred = norms
            for n in deferred:
                do_norm(*n)

    nc.compile()
    return nc


def _get_nc(reps: int = 1):
    if reps not in _CACHE:
        _CACHE[reps] = _build_nc(reps)
    return _CACHE[reps]


def make_in_maps(q, k, v):
    """Host-side shard prep: per-core input dicts (numpy only)."""
    q = np.asarray(q, dtype=np.float32).reshape(B * H, S, D)
    k = np.asarray(k, dtype=np.float32).reshape(B * H, S, D)
    v = np.asarray(v, dtype=np.float32).reshape(B * H, S, D)

    qt = np.ascontiguousarray(q.transpose(0, 2, 1))  # [32, D, S]
    kt = np.ascontiguousarray(k.transpose(0, 2, 1))  # [32, D, S]
    ones = np.ones((B * H, S, 1), dtype=np.float32)
    # [32, S, 65] -> k-tile-major [32, 128, 16*65], bf16
    vp = (
        np.concatenate([v, ones], axis=2)
        .reshape(B * H, KT_N, 128, MV)
        .transpose(0, 2, 1, 3)
        .reshape(B * H, 128, KT_N * MV)
    )
    vp = np.ascontiguousarray(vp).astype(ml_dtypes.bfloat16)

    # wide Schraudolph bias tile: cols [0,1024) plain bias; [1024,1152)
    # causal-mask pattern (rows = k partition, cols = q offset)
    mbw = np.full((128, 512), SCH_BIAS, dtype=np.float32)
    r = np.arange(128)
    mbw[:, 0:128] = np.where(
        r[None, :] >= r[:, None], SCH_BIAS, MASK_NEG
    ).astype(np.float32)

    in_maps = []
    for i in range(N_CORES):
        s = slice(HPC * i, HPC * (i + 1))
        in_maps.append(
            {
                "qt": np.ascontiguousarray(qt[s]),
                "kt": np.ascontiguousarray(kt[s]),
                "vp": np.ascontiguousarray(vp[s]),
                "mbw": mbw,
            }
        )
    return in_maps


def gather_output(results):
    """Assemble full [B, H, S, D] output from per-core o[h, p, qb, d]."""
    oo = np.concatenate(
        [np.asarray(results[i]["o"]) for i in range(N_CORES)], axis=0
    )  # [32, 128, 16, 64]
    # q = 128*qb + p  ->  [32, 16, 128, 64] -> [32, 2048, 64]
    out = oo.transpose(0, 2, 1, 3).reshape(B, H, S, D)
    return np.ascontiguousarray(out)


def kernel(q, k, v):
    nc = _get_nc()
    in_maps = make_in_maps(q, k, v)
    res = bass_utils.run_bass_kernel_spmd(
        nc, in_maps, core_ids=list(range(N_CORES))
    )
    return gather_output(res.results)


# revision 41
# speedup vs baseline: 1.3118x; 1.3118x over previous
"""Causal attention kernel for TRN2, sharded over 8 NeuronCores.

Problem: q,k,v [B=2, H=16, S=2048, D=64] fp32 -> causal softmax(QK^T)V.
Sharding: 32 (batch*head) pairs -> 4 heads per core (head-parallel, no comm).

Per-core algorithm (per head), v2 -- engine-balanced:
  Scores are computed TRANSPOSED per 512-wide q-chunk: st[k, q] via
  matmul(out=sc, lhsT=KT[d, k-tile], rhs=QT[d, q-chunk-slice]), k-tiles
  processed in pairs sharing one [128, 1024] PSUM tile (fp32r, all output
  pieces >= 256 wide so the PE streams 1 col/cycle).
  exp() is split across THREE engines per stage:
    - ACT: exact exp (bf16 out), one strided instruction covering both
      tiles' ACT column shares.
    - DVE + Pool (gpsimd): Schraudolph exp -- one scalar_tensor_tensor
      computing round_i16(score * 128*log2(e) + bias) written through the
      bf16 ex tile's int16 bitcast view; the int16 bits ARE the bf16
      encoding of 2^(x*log2 e) ~ exp(x) (~3% max rel err; tolerance 2e-2).
      The causal mask is FOLDED into the same op: the in1 bias tile holds
      (bias - 3e5) on disallowed diagonal lanes, so the int16 convert
      saturates negative -> bitcast -0.0 -> contributes exactly 0.
  mm2 is q-on-partitions: matmul(acc[q-block 128, 65] +=,
  lhsT=ex[k-tile, q-block], rhs=[V|1][k-tile, 65] bf16): 65-wide moving
  operand at 1 col/cycle (bf16) -- half the PE cost of the 512-wide
  accT formulation -- and the softmax denominator Z lands in acc column
  64 per PARTITION, so normalization is a native per-partition scalar:
  reciprocal_approx_fast on the strided Z view + one broadcast multiply.
  Output is emitted per chunk as o[h][p, qblock, d] (q = 128*qblock + p);
  host reorders.
"""

import numpy as np
import ml_dtypes

import concourse.bacc as bacc
import concourse.tile as tile
from concourse import mybir
from concourse import bass_utils

B, H, S, D = 2, 16, 2048, 64
N_CORES = 8
HPC = (B * H) // N_CORES  # heads per core = 4
F32 = mybir.dt.float32
F32R = mybir.dt.float32r  # full-rate PE matmul mode (TF32-like rounding)
BF16 = mybir.dt.bfloat16
I16 = mybir.dt.int16
Alu = mybir.AluOpType
KT_N = S // 128  # 16 k-tiles per head
MV = D + 1  # V columns + ones column
NCH = 4  # 512-wide q-chunks per head

LOG2E = float(np.log2(np.e))
SCH_SCALE = LOG2E * 128.0
SCH_BIAS = 127.0 * 128.0 - 5.5  # optimal-C Schraudolph bias (bf16 bits)
MASK_NEG = -3.0e5  # saturates int16 negative -> bf16 -0.0

FA = 400  # ACT column share of each full 512 tile segment
WARM_N = 6  # PE p-state warmup matmuls

_CACHE = {}


def _build_nc(reps: int = 1):
    """Build + compile the SPMD bass program (same for every core)."""
    nc = bacc.Bacc(
        "TRN2", target_bir_lowering=False, debug=False, num_devices=N_CORES
    )
    qt = nc.dram_tensor("qt", [HPC, D, S], F32R, kind="ExternalInput").ap()
    kt = nc.dram_tensor("kt", [HPC, D, S], F32R, kind="ExternalInput").ap()
    vp = nc.dram_tensor("vp", [HPC, 128, KT_N * MV], BF16, kind="ExternalInput").ap()
    mbw = nc.dram_tensor("mbw", [128, 512], F32, kind="ExternalInput").ap()
    o = nc.dram_tensor("o", [HPC, 128, KT_N, D], F32, kind="ExternalOutput").ap()

    with tile.TileContext(nc) as tc:
        with (
            tc.tile_pool(name="io", bufs=3) as io_pool,
            tc.tile_pool(name="maskp", bufs=1) as mask_pool,
            tc.tile_pool(name="expa", bufs=12) as expa_pool,
            tc.tile_pool(name="expd", bufs=12) as expd_pool,
            tc.tile_pool(name="outp", bufs=3) as out_pool,
            tc.tile_pool(name="zp", bufs=4) as z_pool,
            tc.tile_pool(name="scores", bufs=3, space="PSUM") as sc_pool,
            tc.tile_pool(name="acc", bufs=2, space="PSUM") as acc_pool,
        ):
            mbw_s = mask_pool.tile([128, 512], F32)

            # PE p-state warmup: the tensor engine needs ~3us of continuous
            # execution to reach full clock (any idle resets it).  Run dummy
            # matmuls on a zeroed tile from t~0.4us so the ramp completes
            # while the first input DMAs are still in flight; overshoot so
            # the first real mm1 starts back-to-back (a gap would reset).
            wz = mask_pool.tile([D, 512], F32)
            nc.gpsimd.memset(wz[:], 0.0)
            # tiny dummy activation so the Exp table load happens at t~0,
            # not chained behind the first real activation's dep waits
            wact = mask_pool.tile([D, 1], BF16)
            nc.scalar.activation(
                wact[:], wz[:, 0:1],
                mybir.ActivationFunctionType.Exp,
            )
            wp = sc_pool.tile([128, 1024], F32, tag="sc")
            for _w in range(WARM_N):
                nc.tensor.matmul(
                    wp[:, 512 * (_w % 2) : 512 * (_w % 2) + 512],
                    lhsT=wz[:, 0:128].bitcast(F32R),
                    rhs=wz[:].bitcast(F32R),
                    start=True,
                    stop=True,
                )

            io_of = {}  # (rep, h) -> (qt_s, kt_s, vp_s)

            def get_io(rep, h):
                if (rep, h) not in io_of:
                    qt_s = io_pool.tile([D, S], F32R, tag="qt")
                    kt_s = io_pool.tile([D, S], F32R, tag="kt")
                    vp_s = io_pool.tile([128, KT_N * MV], BF16, tag="vp")
                    if rep == 0 and h == 0:
                        # staged loads ordered by first use (HWDGE is serial)
                        # first chunk is c=1: tiles 0..7, q [512,1024)
                        nc.gpsimd.dma_start(mbw_s[:], mbw[:])
                        nc.gpsimd.dma_start(vp_s[:], vp[h])
                        nc.sync.dma_start(kt_s[:, 0:1024], kt[h][:, 0:1024])
                        nc.sync.dma_start(qt_s[:, 512:1024], qt[h][:, 512:1024])
                        nc.sync.dma_start(qt_s[:, 1024:S], qt[h][:, 1024:S])
                        nc.sync.dma_start(kt_s[:, 1024:S], kt[h][:, 1024:S])
                        nc.sync.dma_start(qt_s[:, 0:512], qt[h][:, 0:512])
                    else:
                        nc.sync.dma_start(kt_s[:], kt[h])
                        nc.sync.dma_start(qt_s[:], qt[h])
                        nc.sync.dma_start(vp_s[:], vp[h])
                    io_of[(rep, h)] = (qt_s, kt_s, vp_s)
                return io_of[(rep, h)]

            acc_of = {}  # (rep, h, c) -> acc tile [128, 4*65]

            # ---- stage emission -------------------------------------------
            # stage = (rep, h, c, s): chunk c covers q [512c, 512c+512),
            # pair s covers k-tiles (2s, 2s+1); s in 0..2c+1.

            def emit_mm1(rep, h, c, s):
                qt_s, kt_s, vp_s = get_io(rep, h)
                if (rep, h, c) not in acc_of:
                    acc_of[(rep, h, c)] = acc_pool.tile(
                        [128, 4 * MV], F32, tag="acc", name=f"acc_{rep}_{h}_{c}"
                    )
                sc = sc_pool.tile([128, 1024], F32, tag="sc")
                for ti, t in enumerate((2 * s, 2 * s + 1)):
                    ls = max(0, 128 * t - 512 * c)  # local q start
                    ms = min(ls, 256)  # >=256-wide piece for fp32r full rate
                    base = 512 * ti
                    nc.tensor.matmul(
                        sc[:, base + ms : base + 512],
                        lhsT=kt_s[:, 128 * t : 128 * t + 128],
                        rhs=qt_s[:, 512 * c + ms : 512 * c + 512],
                        start=True,
                        stop=True,
                    )
                return sc

            def emit_exp(rep, h, c, s, sc):
                """Split exp across ACT (exact) and DVE (Schraudolph).

                ACT and DVE write SEPARATE bf16 tiles (shared-tile writes get
                spuriously serialized by bounding-box subtile dep tracking);
                ownership is 128-col-block aligned so each mm2 lhsT slice
                lives wholly in one tile.  Returns (ex_a, ex_d, owners) where
                owners[bi] is True if block bi (col bi*128) is ACT's."""
                ex_a = expa_pool.tile([128, 1024], BF16, tag="exa")
                ex_d = expd_pool.tile([128, 1024], BF16, tag="exd")
                owners = [False] * 8

                def act(lo, w):
                    for bi in range(lo // 128, (lo + w) // 128):
                        owners[bi] = True
                    nc.scalar.activation(
                        ex_a[:, lo : lo + w],
                        sc[:, lo : lo + w],
                        mybir.ActivationFunctionType.Exp,
                    )

                def stt(lo, w, masked):
                    # DVE Schraudolph (GPSIMD cannot access PSUM)
                    boff = 0 if masked else 128
                    nc.vector.scalar_tensor_tensor(
                        ex_d[:, lo : lo + w].bitcast(I16),
                        sc[:, lo : lo + w],
                        SCH_SCALE,
                        mbw_s[:, boff : boff + w],
                        Alu.mult,
                        Alu.add,
                    )

                def act2(lo):
                    # ACT on [lo,lo+256) of BOTH tile segments via one 3D AP
                    for bi in (lo // 128, lo // 128 + 1, 4 + lo // 128, 5 + lo // 128):
                        owners[bi] = True
                    av = ex_a[:].rearrange("p (t q) -> p t q", t=2)
                    sv = sc[:].rearrange("p (t q) -> p t q", t=2)
                    nc.scalar.activation(
                        av[:, :, lo : lo + 256],
                        sv[:, :, lo : lo + 256],
                        mybir.ActivationFunctionType.Exp,
                    )

                # NOTE: always emit the DVE read of sc BEFORE the ACT read.
                # Tile chains same-tile readers in emission order (so the
                # sc-slot WAR waits only on the last reader); alternating
                # the order zigzag-chains ACT<->DVE into lockstep.
                # Splits keep ACT ~= DVE busy WITHIN every stage type.
                if s < 2 * c:  # full pair
                    if s % 2 == 0:
                        stt(640, 384, False)
                        act(0, 640)
                    else:
                        stt(0, 384, False)
                        act(384, 640)
                elif s == 2 * c:  # tiles 4c (diag at 0), 4c+1 (diag at 128)
                    stt(0, 256, True)  # tile0 diag + one full block
                    stt(512 + 128, 128, True)  # tile1 diag block
                    act2(256)  # [256,512) + [768,1024)
                else:  # s == 2c+1: tiles 4c+2 (diag at 256), 4c+3 (diag at 384)
                    stt(256, 128, True)
                    stt(512 + 384, 128, True)
                    act(384, 128)
                return ex_a, ex_d, owners

            ex_of = {}  # (rep, h, c, s) -> ex tile (alive for the chunk)

            def emit_mm2_block(rep, h, c, b):
                """Emit q-block b's FULL accumulation chain consecutively.

                PSUM accumulation groups sharing a bank must not interleave
                (hw corrupts the early-stopping group), so each block's
                matmuls run back-to-back once all its k-tiles' exp is done."""
                _, _, vp_s = get_io(rep, h)
                acc = acc_of[(rep, h, c)]
                tmax = 4 * c + b
                for t in range(tmax + 1):
                    ex_a, ex_d, owners = ex_of[(rep, h, c, t // 2)]
                    bi = 4 * (t % 2) + b
                    ex = ex_a if owners[bi] else ex_d
                    nc.tensor.matmul(
                        acc[:, MV * b : MV * b + MV],
                        lhsT=ex[:, 128 * bi : 128 * bi + 128],
                        rhs=vp_s[:, MV * t : MV * t + MV],
                        start=(t == 0),
                        stop=(t == tmax),
                    )

            def do_norm(rep, h, c, b0=0, nb=4, pop=True):
                acc = acc_of.pop((rep, h, c)) if pop else acc_of[(rep, h, c)]
                zs = z_pool.tile([128, nb], F32, tag=f"zs{nb}")
                nc.vector.reciprocal_approx_fast(
                    zs[:], acc[:, MV * b0 + D : MV * (b0 + nb) : MV]
                )
                o_s = out_pool.tile([128, nb * D], F32, tag=f"os{nb}")
                accv = acc[:, MV * b0 : MV * (b0 + nb)].rearrange(
                    "p (b c) -> p b c", b=nb
                )
                nc.vector.tensor_tensor(
                    o_s[:].rearrange("p (b c) -> p b c", b=nb),
                    accv[:, :, 0:D],
                    zs[:].unsqueeze(2).broadcast_to([128, nb, D]),
                    Alu.mult,
                )
                nc.sync.dma_start(
                    o[h][:, 4 * c + b0 : 4 * c + b0 + nb, :], o_s[:]
                )

            # ---- flattened software pipeline ------------------------------
            # Flat chunk order: each head's small DVE-heavy chunk 0 is
            # deferred until after the NEXT head's first chunk, so its ACT
            # trough is filled by that head's ACT-rich full stages.
            chunk_seq = []
            units = [(r, h) for r in range(reps) for h in range(HPC)]
            for idx, (r, h) in enumerate(units):
                chunk_seq.append((r, h, 1))
                if idx > 0:
                    chunk_seq.append((*units[idx - 1], 0))
                chunk_seq.append((r, h, 2))
                chunk_seq.append((r, h, 3))
            chunk_seq.append((*units[-1], 0))
            stages = []
            for r, h, c in chunk_seq:
                for s in range(2 * c + 2):
                    stages.append((r, h, c, s))

            def compute_stage(pst, psc):
                rep_, h_, c_, s_ = pst
                ex_of[(rep_, h_, c_, s_)] = emit_exp(*pst, psc)
                if s_ == 2 * c_:
                    emit_mm2_block(rep_, h_, c_, 0)
                    emit_mm2_block(rep_, h_, c_, 1)
                elif s_ == 2 * c_ + 1:
                    emit_mm2_block(rep_, h_, c_, 2)
                    emit_mm2_block(rep_, h_, c_, 3)
                    for s2 in range(2 * c_ + 2):
                        del ex_of[(rep_, h_, c_, s2)]
                    return [(rep_, h_, c_)]
                return []

            # mm1 runs TWO stages ahead of exp/mm2 so the next chunk's
            # scores are already in PSUM while this chunk's block chains
            # and norm drain -- keeps ACT/DVE fed across chunk boundaries.
            pending = []  # [(stage, sc)] not yet computed
            deferred = []  # [(due_step, norm)] chunk norms deferred 3 steps
            step = 0

            def run_deferred(final=False):
                while deferred and (final or deferred[0][0] <= step):
                    do_norm(*deferred.pop(0)[1])

            for i, st in enumerate(stages):
                rep_, h_, c_, s_ = st
                if c_ == 3 and s_ == 0:  # third chunk in [1,2,3,0] order
                    nxt = (rep_, h_ + 1) if h_ + 1 < HPC else (rep_ + 1, 0)
                    if nxt[0] < reps:
                        get_io(*nxt)
                pending.append((st, emit_mm1(*st)))
                if len(pending) > 2:
                    step += 1
                    norms = compute_stage(*pending.pop(0))
                    run_deferred()
                    deferred += [(step + 3, n) for n in norms]
            for p in pending:
                step += 1
                norms = compute_stage(*p)
                run_deferred()
                deferred += [(step + 3, n) for n in norms]
            run_deferred(final=True)

    nc.compile()
    return nc


def _get_nc(reps: int = 1):
    if reps not in _CACHE:
        _CACHE[reps] = _build_nc(reps)
    return _CACHE[reps]


def make_in_maps(q, k, v):
    """Host-side shard prep: per-core input dicts (numpy only)."""
    q = np.asarray(q, dtype=np.float32).reshape(B * H, S, D)
    k = np.asarray(k, dtype=np.float32).reshape(B * H, S, D)
    v = np.asarray(v, dtype=np.float32).reshape(B * H, S, D)

    qt = np.ascontiguousarray(q.transpose(0, 2, 1))  # [32, D, S]
    kt = np.ascontiguousarray(k.transpose(0, 2, 1))  # [32, D, S]
    ones = np.ones((B * H, S, 1), dtype=np.float32)
    # [32, S, 65] -> k-tile-major [32, 128, 16*65], bf16
    vp = (
        np.concatenate([v, ones], axis=2)
        .reshape(B * H, KT_N, 128, MV)
        .transpose(0, 2, 1, 3)
        .reshape(B * H, 128, KT_N * MV)
    )
    vp = np.ascontiguousarray(vp).astype(ml_dtypes.bfloat16)

    # wide Schraudolph bias tile: cols [0,1024) plain bias; [1024,1152)
    # causal-mask pattern (rows = k partition, cols = q offset)
    mbw = np.full((128, 512), SCH_BIAS, dtype=np.float32)
    r = np.arange(128)
    mbw[:, 0:128] = np.where(
        r[None, :] >= r[:, None], SCH_BIAS, MASK_NEG
    ).astype(np.float32)

    in_maps = []
    for i in range(N_CORES):
        s = slice(HPC * i, HPC * (i + 1))
        in_maps.append(
            {
                "qt": np.ascontiguousarray(qt[s]),
                "kt": np.ascontiguousarray(kt[s]),
                "vp": np.ascontiguousarray(vp[s]),
                "mbw": mbw,
            }
        )
    return in_maps


def gather_output(results):
    """Assemble full [B, H, S, D] output from per-core o[h, p, qb, d]."""
    oo = np.concatenate(
        [np.asarray(results[i]["o"]) for i in range(N_CORES)], axis=0
    )  # [32, 128, 16, 64]
    # q = 128*qb + p  ->  [32, 16, 128, 64] -> [32, 2048, 64]
    out = oo.transpose(0, 2, 1, 3).reshape(B, H, S, D)
    return np.ascontiguousarray(out)


def kernel(q, k, v):
    nc = _get_nc()
    in_maps = make_in_maps(q, k, v)
    res = bass_utils.run_bass_kernel_spmd(
        nc, in_maps, core_ids=list(range(N_CORES))
    )
    return gather_output(res.results)


# revision 47
# speedup vs baseline: 1.5011x; 1.1443x over previous
"""Causal attention kernel for TRN2, sharded over 8 NeuronCores.

Problem: q,k,v [B=2, H=16, S=2048, D=64] fp32 -> causal softmax(QK^T)V.
Sharding: 32 (batch*head) pairs -> 4 heads per core (head-parallel, no comm).

Per-core algorithm (per head), v2 -- engine-balanced:
  Scores are computed TRANSPOSED per 512-wide q-chunk: st[k, q] via
  matmul(out=sc, lhsT=KT[d, k-tile], rhs=QT[d, q-chunk-slice]), k-tiles
  processed in pairs sharing one [128, 1024] PSUM tile (fp32r, all output
  pieces >= 256 wide so the PE streams 1 col/cycle).
  exp() is split across THREE engines per stage:
    - ACT: exact exp (bf16 out), one strided instruction covering both
      tiles' ACT column shares.
    - DVE + Pool (gpsimd): Schraudolph exp -- one scalar_tensor_tensor
      computing round_i16(score * 128*log2(e) + bias) written through the
      bf16 ex tile's int16 bitcast view; the int16 bits ARE the bf16
      encoding of 2^(x*log2 e) ~ exp(x) (~3% max rel err; tolerance 2e-2).
      The causal mask is FOLDED into the same op: the in1 bias tile holds
      (bias - 3e5) on disallowed diagonal lanes, so the int16 convert
      saturates negative -> bitcast -0.0 -> contributes exactly 0.
  mm2 is q-on-partitions: matmul(acc[q-block 128, 65] +=,
  lhsT=ex[k-tile, q-block], rhs=[V|1][k-tile, 65] bf16): 65-wide moving
  operand at 1 col/cycle (bf16) -- half the PE cost of the 512-wide
  accT formulation -- and the softmax denominator Z lands in acc column
  64 per PARTITION, so normalization is a native per-partition scalar:
  reciprocal_approx_fast on the strided Z view + one broadcast multiply.
  Output is emitted per chunk as o[h][p, qblock, d] (q = 128*qblock + p);
  host reorders.
"""

import numpy as np
import ml_dtypes

import concourse.bacc as bacc
import concourse.tile as tile
from concourse import mybir
from concourse import bass_utils

B, H, S, D = 2, 16, 2048, 64
N_CORES = 8
HPC = (B * H) // N_CORES  # heads per core = 4
F32 = mybir.dt.float32
F32R = mybir.dt.float32r  # full-rate PE matmul mode (TF32-like rounding)
BF16 = mybir.dt.bfloat16
I16 = mybir.dt.int16
Alu = mybir.AluOpType
KT_N = S // 128  # 16 k-tiles per head
MV = D + 1  # V columns + ones column
NCH = 4  # 512-wide q-chunks per head

LOG2E = float(np.log2(np.e))
SCH_SCALE = LOG2E * 128.0
SCH_BIAS = 127.0 * 128.0 - 5.5  # optimal-C Schraudolph bias (bf16 bits)
MASK_NEG = -3.0e5  # saturates int16 negative -> bf16 -0.0

FA = 400  # ACT column share of each full 512 tile segment
WARM_N = 6  # PE p-state warmup matmuls

_CACHE = {}


def _build_nc(reps: int = 1):
    """Build + compile the SPMD bass program (same for every core)."""
    nc = bacc.Bacc(
        "TRN2", target_bir_lowering=False, debug=False, num_devices=N_CORES
    )
    qt = nc.dram_tensor("qt", [HPC, D, S], F32R, kind="ExternalInput").ap()
    kt = nc.dram_tensor("kt", [HPC, D, S], F32R, kind="ExternalInput").ap()
    vp = nc.dram_tensor("vp", [HPC, 128, KT_N * MV], BF16, kind="ExternalInput").ap()
    mbw = nc.dram_tensor("mbw", [128, 640], F32, kind="ExternalInput").ap()
    o = nc.dram_tensor("o", [HPC, 128, KT_N, D], F32, kind="ExternalOutput").ap()

    with tile.TileContext(nc) as tc:
        with (
            tc.tile_pool(name="io", bufs=3) as io_pool,
            tc.tile_pool(name="maskp", bufs=1) as mask_pool,
            tc.tile_pool(name="expa", bufs=12) as expa_pool,
            tc.tile_pool(name="expd", bufs=12) as expd_pool,
            tc.tile_pool(name="outp", bufs=3) as out_pool,
            tc.tile_pool(name="zp", bufs=4) as z_pool,
            tc.tile_pool(name="sca", bufs=3, space="PSUM") as sca_pool,
            tc.tile_pool(name="scd", bufs=3, space="PSUM") as scd_pool,
            tc.tile_pool(name="acc", bufs=2, space="PSUM") as acc_pool,
        ):
            mbw_s = mask_pool.tile([128, 640], F32)

            # PE p-state warmup: the tensor engine needs ~3us of continuous
            # execution to reach full clock (any idle resets it).  Run dummy
            # matmuls on a zeroed tile from t~0.4us so the ramp completes
            # while the first input DMAs are still in flight; overshoot so
            # the first real mm1 starts back-to-back (a gap would reset).
            wz = mask_pool.tile([D, 512], F32)
            nc.gpsimd.memset(wz[:], 0.0)
            # tiny dummy activation so the Exp table load happens at t~0,
            # not chained behind the first real activation's dep waits
            wact = mask_pool.tile([D, 1], BF16)
            nc.scalar.activation(
                wact[:], wz[:, 0:1],
                mybir.ActivationFunctionType.Exp,
            )
            wp = sca_pool.tile([128, 512], F32, tag="sca")
            for _w in range(WARM_N):
                nc.tensor.matmul(
                    wp[:],
                    lhsT=wz[:, 0:128].bitcast(F32R),
                    rhs=wz[:].bitcast(F32R),
                    start=True,
                    stop=True,
                )

            io_of = {}  # (rep, h) -> (qt_s, kt_s, vp_s)

            def get_io(rep, h):
                if (rep, h) not in io_of:
                    qt_s = io_pool.tile([D, S], F32R, tag="qt")
                    kt_s = io_pool.tile([D, S], F32R, tag="kt")
                    vp_s = io_pool.tile([128, KT_N * MV], BF16, tag="vp")
                    if rep == 0 and h == 0:
                        # staged loads ordered by first use (HWDGE is serial)
                        # first chunk is c=1: tiles 0..7, q [512,1024)
                        nc.gpsimd.dma_start(mbw_s[:], mbw[:])
                        nc.gpsimd.dma_start(vp_s[:], vp[h])
                        nc.sync.dma_start(kt_s[:, 0:1024], kt[h][:, 0:1024])
                        nc.sync.dma_start(qt_s[:, 512:1024], qt[h][:, 512:1024])
                        nc.sync.dma_start(qt_s[:, 1024:S], qt[h][:, 1024:S])
                        nc.sync.dma_start(kt_s[:, 1024:S], kt[h][:, 1024:S])
                        nc.sync.dma_start(qt_s[:, 0:512], qt[h][:, 0:512])
                    else:
                        nc.sync.dma_start(kt_s[:], kt[h])
                        nc.sync.dma_start(qt_s[:], qt[h])
                        nc.sync.dma_start(vp_s[:], vp[h])
                    io_of[(rep, h)] = (qt_s, kt_s, vp_s)
                return io_of[(rep, h)]

            acc_of = {}  # (rep, h, c) -> acc tile [128, 4*65]

            # ---- stage emission -------------------------------------------
            # stage = (rep, h, c, s): chunk c covers q [512c, 512c+512),
            # pair s covers k-tiles (2s, 2s+1); s in 0..2c+1.
            #
            # Each 512-col k-tile segment gets its OWN single-bank PSUM
            # tile, owned by ONE exp engine.  Tile inserts reader->reader
            # chains per tile; with one reader per tile the sc-slot WAR
            # latency loop (act -> mm1(s+3) -> stt -> act) disappears.
            # ACT's >512-col share comes from periodic both-ACT stages.
            # seg plan entries: (pool, ms, [stt ranges], [act ranges]);
            # ranges are tile-local (lo, w, masked).
            _FBASE = {1: 0, 2: 2, 3: 6}

            def stage_plan(c, s):
                if s < 2 * c:  # full pair
                    pat = (_FBASE[c] + s) % 4
                    A = ("a", 0, [], [(0, 512)])
                    Dv = ("d", 0, [(0, 512, False)], [])
                    return [(A, Dv), (Dv, A), (A, A), (Dv, A)][pat]
                if s == 2 * c:
                    return (
                        ("d", 0, [(0, 256, True)], [(256, 256)]),
                        ("a", 128, [(128, 128, True)], [(256, 256)]),
                    )
                return (
                    ("a", 256, [(256, 128, True)], [(384, 128)]),
                    ("d", 256, [(384, 128, True)], []),
                )

            def emit_mm1(rep, h, c, s):
                qt_s, kt_s, vp_s = get_io(rep, h)
                if (rep, h, c) not in acc_of:
                    acc_of[(rep, h, c)] = acc_pool.tile(
                        [128, 4 * MV], F32, tag="acc", name=f"acc_{rep}_{h}_{c}"
                    )
                plan = stage_plan(c, s)
                scs = []
                for ti, t in enumerate((2 * s, 2 * s + 1)):
                    pool, ms = plan[ti][0], plan[ti][1]
                    sct = (sca_pool if pool == "a" else scd_pool).tile(
                        [128, 512], F32, tag="sc" + pool
                    )
                    nc.tensor.matmul(
                        sct[:, ms:512],
                        lhsT=kt_s[:, 128 * t : 128 * t + 128],
                        rhs=qt_s[:, 512 * c + ms : 512 * c + 512],
                        start=True,
                        stop=True,
                    )
                    scs.append(sct)
                return scs

            def emit_exp(rep, h, c, s, scs):
                """Per-segment exp: ACT exact / DVE Schraudolph, writing
                separate bf16 tiles; owners[bi] True => block is ACT's."""
                ex_a = expa_pool.tile([128, 1024], BF16, tag="exa")
                ex_d = expd_pool.tile([128, 1024], BF16, tag="exd")
                owners = [False] * 8
                plan = stage_plan(c, s)
                # DVE reads first (chain tiles: stt then act)
                for ti in range(2):
                    sct = scs[ti]
                    for lo, w, masked in plan[ti][2]:
                        boff = 0 if masked else 128
                        nc.vector.scalar_tensor_tensor(
                            ex_d[:, 512 * ti + lo : 512 * ti + lo + w].bitcast(I16),
                            sct[:, lo : lo + w],
                            SCH_SCALE,
                            mbw_s[:, boff : boff + w],
                            Alu.mult,
                            Alu.add,
                        )
                for ti in range(2):
                    sct = scs[ti]
                    for lo, w in plan[ti][3]:
                        for bi in range(lo // 128, (lo + w) // 128):
                            owners[4 * ti + bi] = True
                        nc.scalar.activation(
                            ex_a[:, 512 * ti + lo : 512 * ti + lo + w],
                            sct[:, lo : lo + w],
                            mybir.ActivationFunctionType.Exp,
                        )
                return ex_a, ex_d, owners

            ex_of = {}  # (rep, h, c, s) -> ex tile (alive for the chunk)

            def emit_mm2_block(rep, h, c, b):
                """Emit q-block b's FULL accumulation chain consecutively.

                PSUM accumulation groups sharing a bank must not interleave
                (hw corrupts the early-stopping group), so each block's
                matmuls run back-to-back once all its k-tiles' exp is done."""
                _, _, vp_s = get_io(rep, h)
                acc = acc_of[(rep, h, c)]
                tmax = 4 * c + b
                for t in range(tmax + 1):
                    ex_a, ex_d, owners = ex_of[(rep, h, c, t // 2)]
                    bi = 4 * (t % 2) + b
                    ex = ex_a if owners[bi] else ex_d
                    nc.tensor.matmul(
                        acc[:, MV * b : MV * b + MV],
                        lhsT=ex[:, 128 * bi : 128 * bi + 128],
                        rhs=vp_s[:, MV * t : MV * t + MV],
                        start=(t == 0),
                        stop=(t == tmax),
                    )

            def do_norm(rep, h, c, b0=0, nb=4, pop=True):
                acc = acc_of.pop((rep, h, c)) if pop else acc_of[(rep, h, c)]
                zs = z_pool.tile([128, nb], F32, tag=f"zs{nb}")
                nc.vector.reciprocal_approx_fast(
                    zs[:], acc[:, MV * b0 + D : MV * (b0 + nb) : MV]
                )
                o_s = out_pool.tile([128, nb * D], F32, tag=f"os{nb}")
                accv = acc[:, MV * b0 : MV * (b0 + nb)].rearrange(
                    "p (b c) -> p b c", b=nb
                )
                nc.vector.tensor_tensor(
                    o_s[:].rearrange("p (b c) -> p b c", b=nb),
                    accv[:, :, 0:D],
                    zs[:].unsqueeze(2).broadcast_to([128, nb, D]),
                    Alu.mult,
                )
                nc.sync.dma_start(
                    o[h][:, 4 * c + b0 : 4 * c + b0 + nb, :], o_s[:]
                )

            # ---- flattened software pipeline ------------------------------
            # Flat chunk order: each head's small DVE-heavy chunk 0 is
            # deferred until after the NEXT head's first chunk, so its ACT
            # trough is filled by that head's ACT-rich full stages.
            chunk_seq = []
            units = [(r, h) for r in range(reps) for h in range(HPC)]
            for idx, (r, h) in enumerate(units):
                chunk_seq.append((r, h, 1))
                if idx > 0:
                    chunk_seq.append((*units[idx - 1], 0))
                chunk_seq.append((r, h, 2))
                chunk_seq.append((r, h, 3))
            chunk_seq.append((*units[-1], 0))
            stages = []
            for r, h, c in chunk_seq:
                for s in range(2 * c + 2):
                    stages.append((r, h, c, s))

            def compute_stage(pst, psc):
                rep_, h_, c_, s_ = pst
                ex_of[(rep_, h_, c_, s_)] = emit_exp(*pst, psc)
                if s_ == 2 * c_:
                    emit_mm2_block(rep_, h_, c_, 0)
                    emit_mm2_block(rep_, h_, c_, 1)
                elif s_ == 2 * c_ + 1:
                    emit_mm2_block(rep_, h_, c_, 2)
                    emit_mm2_block(rep_, h_, c_, 3)
                    for s2 in range(2 * c_ + 2):
                        del ex_of[(rep_, h_, c_, s2)]
                    return [(rep_, h_, c_)]
                return []

            # mm1 runs TWO stages ahead of exp/mm2 so the next chunk's
            # scores are already in PSUM while this chunk's block chains
            # and norm drain -- keeps ACT/DVE fed across chunk boundaries.
            pending = []  # [(stage, sc)] not yet computed
            deferred = []  # [(due_step, norm)] chunk norms deferred 3 steps
            step = 0

            def run_deferred(final=False):
                while deferred and (final or deferred[0][0] <= step):
                    do_norm(*deferred.pop(0)[1])

            for i, st in enumerate(stages):
                rep_, h_, c_, s_ = st
                if c_ == 3 and s_ == 0:  # third chunk in [1,2,3,0] order
                    nxt = (rep_, h_ + 1) if h_ + 1 < HPC else (rep_ + 1, 0)
                    if nxt[0] < reps:
                        get_io(*nxt)
                pending.append((st, emit_mm1(*st)))
                if len(pending) > 2:
                    step += 1
                    norms = compute_stage(*pending.pop(0))
                    run_deferred()
                    deferred += [(step + 3, n) for n in norms]
            for p in pending:
                step += 1
                norms = compute_stage(*p)
                run_deferred()
                deferred += [(step + 3, n) for n in norms]
            run_deferred(final=True)

    nc.compile()
    return nc


def _get_nc(reps: int = 1):
    if reps not in _CACHE:
        _CACHE[reps] = _build_nc(reps)
    return _CACHE[reps]


def make_in_maps(q, k, v):
    """Host-side shard prep: per-core input dicts (numpy only)."""
    q = np.asarray(q, dtype=np.float32).reshape(B * H, S, D)
    k = np.asarray(k, dtype=np.float32).reshape(B * H, S, D)
    v = np.asarray(v, dtype=np.float32).reshape(B * H, S, D)

    qt = np.ascontiguousarray(q.transpose(0, 2, 1))  # [32, D, S]
    kt = np.ascontiguousarray(k.transpose(0, 2, 1))  # [32, D, S]
    ones = np.ones((B * H, S, 1), dtype=np.float32)
    # [32, S, 65] -> k-tile-major [32, 128, 16*65], bf16
    vp = (
        np.concatenate([v, ones], axis=2)
        .reshape(B * H, KT_N, 128, MV)
        .transpose(0, 2, 1, 3)
        .reshape(B * H, 128, KT_N * MV)
    )
    vp = np.ascontiguousarray(vp).astype(ml_dtypes.bfloat16)

    # wide Schraudolph bias tile: cols [0,1024) plain bias; [1024,1152)
    # causal-mask pattern (rows = k partition, cols = q offset)
    mbw = np.full((128, 640), SCH_BIAS, dtype=np.float32)
    r = np.arange(128)
    mbw[:, 0:128] = np.where(
        r[None, :] >= r[:, None], SCH_BIAS, MASK_NEG
    ).astype(np.float32)

    in_maps = []
    for i in range(N_CORES):
        s = slice(HPC * i, HPC * (i + 1))
        in_maps.append(
            {
                "qt": np.ascontiguousarray(qt[s]),
                "kt": np.ascontiguousarray(kt[s]),
                "vp": np.ascontiguousarray(vp[s]),
                "mbw": mbw,
            }
        )
    return in_maps


def gather_output(results):
    """Assemble full [B, H, S, D] output from per-core o[h, p, qb, d]."""
    oo = np.concatenate(
        [np.asarray(results[i]["o"]) for i in range(N_CORES)], axis=0
    )  # [32, 128, 16, 64]
    # q = 128*qb + p  ->  [32, 16, 128, 64] -> [32, 2048, 64]
    out = oo.transpose(0, 2, 1, 3).reshape(B, H, S, D)
    return np.ascontiguousarray(out)


def kernel(q, k, v):
    nc = _get_nc()
    in_maps = make_in_maps(q, k, v)
    res = bass_utils.run_bass_kernel_spmd(
        nc, in_maps, core_ids=list(range(N_CORES))
    )
    return gather_output(res.results)
